# revision 13
# baseline (speedup 1.0000x reference)
"""Additive-attention (ContentAttender) Bass kernel for 8 TRN2 NeuronCores.

Problem: B=4, NQ=512, NK=512, D=128, H=32
  logits[b,q,k] = w2 . tanh(qh[b,q] + kh[b,k] + b1) + b2
  out = softmax_k(logits) @ keys

Sharding: data-parallel over (batch x query-half) -> 8 cores, each core
handles one batch's 256 queries vs all 512 keys. No collectives.

Method: the O(NQ*NK*H) tanh tensor is never materialized. Since the score
is a function of a SUM (qh + kh), expand tanh in a separable trig basis:
  tanh(s) ~= sum_m c_m sin(w_m s),  sin(w(a+b)) = sin(wa)cos(wb)+cos(wa)sin(wb)
(M=4 free-fitted frequencies, end-to-end rel err ~2.4e-3, at the bf16
floor). Each core then only evaluates sin/cos features on the small
qh [256,32] / kh [512,32] tensors and contracts the 2M*H=256 feature dim
on the TensorEngine. Phases are built by PE matmuls with omega-prescaled
replicated weights (fp32 PSUM), range-reduced into [-pi,pi] by single
DVE add_range_wrap ops (HW Sin is only accurate to ~|3.3|; cos-tiles get
the +pi/2 as the wrap shift), activated by ACT Sin (b1 folded into the
per-partition bias). Logits come out TRANSPOSED (k on partitions), so
softmax needs no transposes: the rowsum is a ones-column appended to the
keys in the context matmul moving operand. Input DMAs are issued from
five different engine queues so their ~0.6us issue slots overlap; the
Sin->Exp ACT table switch (~1.3us, unavoidable: no table set holds both)
overlaps the logits matmuls.
"""

import contextlib

import numpy as np
import ml_dtypes

import concourse.bass as bass  # noqa: F401
import concourse.mybir as mybir
import concourse.tile as tile
from concourse import bacc
from concourse.bass_utils import run_bass_kernel_spmd

F32 = mybir.dt.float32
BF16 = mybir.dt.bfloat16
AF = mybir.ActivationFunctionType

B, NQ, NK, D, H = 4, 512, 512, 128, 32
NQC = NQ // 2          # queries per core = 256
M = 4                  # trig terms; feature dim = 2*M*H = 256

# fitted tanh(s) ~= sum_m COEF[m] * sin(OMEGA[m] * s)
OMEGA = np.array([0.58658092, 0.58659907, 1.83957819, 3.31890976])
COEF = np.array([1.15549101, -0.0317051, 0.14888519, 0.01860145])

PI = float(np.pi)

_CACHED_NC = None


def _build_nc():
    nc = bacc.Bacc("TRN2", target_bir_lowering=False, debug=False)

    wmat = nc.declare_dram_parameter("wmat", [128, 2 * D], BF16, isOutput=False)
    kTp = nc.declare_dram_parameter("kT", [128, NK], BF16, isOutput=False)
    qTp = nc.declare_dram_parameter("qT", [128, NQC], BF16, isOutput=False)
    kctxp = nc.declare_dram_parameter("kctx", [128, 4 * 129], BF16, isOutput=False)
    vecsp = nc.declare_dram_parameter("vecs", [128, 2], F32, isOutput=False)
    out = nc.declare_dram_parameter("out", [NQC, D], F32, isOutput=True)

    with tile.TileContext(nc) as tc, contextlib.ExitStack() as ctx:
        cpool = ctx.enter_context(tc.tile_pool(name="consts", bufs=1))
        wpool = ctx.enter_context(tc.tile_pool(name="wraps", bufs=1))
        fpool = ctx.enter_context(tc.tile_pool(name="feats", bufs=1))
        epool = ctx.enter_context(tc.tile_pool(name="softmax", bufs=1))
        ps_b = ctx.enter_context(tc.tile_pool(name="ps_b", bufs=1, space="PSUM"))
        ps_a = ctx.enter_context(tc.tile_pool(name="ps_a", bufs=1, space="PSUM"))
        ps_l = ctx.enter_context(tc.tile_pool(name="ps_l", bufs=1, space="PSUM"))
        ps_t = ctx.enter_context(tc.tile_pool(name="ps_t", bufs=1, space="PSUM"))

        # input DMAs issued from three different queues so they overlap
        # (only sync/scalar/gpsimd may issue DMAs)
        kT = cpool.tile([128, NK], BF16, tag="kT")
        nc.sync.dma_start(kT[:], kTp[:])
        wm = cpool.tile([128, 2 * D], BF16, tag="wm")
        nc.scalar.dma_start(wm[:], wmat[:])
        qT = cpool.tile([128, NQC], BF16, tag="qT")
        nc.gpsimd.dma_start(qT[:], qTp[:])
        vecs = cpool.tile([128, 2], F32, tag="vecs")
        nc.gpsimd.dma_start(vecs[:], vecsp[:])
        kctx = cpool.tile([128, 4 * 129], BF16, tag="kctx")
        nc.gpsimd.dma_start(kctx[:], kctxp[:])

        WkO = wm[:, 0:D]
        WqO = wm[:, D : 2 * D]
        cw = vecs[:, 0:1]
        biasA = vecs[:, 1:2]

        # phases: PB[(m,h), k] = omega_m*kh[k,h]; PA[(m,h), q] = omega_m*qh[q,h]
        # PB in two halves so the first wrap starts earlier
        PB = ps_b.tile([128, NK], F32, tag="PB", name="PB")
        nc.tensor.matmul(PB[:, 0:256], WkO, kT[:, 0:256], start=True, stop=True)
        nc.tensor.matmul(PB[:, 256:512], WkO, kT[:, 256:512], start=True, stop=True)
        PA = ps_a.tile([128, NQC], F32, tag="PA", name="PA")
        nc.tensor.matmul(PA[:], WqO, qT, start=True, stop=True)

        # range-reduce into [-pi, pi]; cos-tiles get +pi/2 shift pre-wrap.
        # b-side wraps first so the ACT sin chain BS->BC->A never idles.
        WBS = wpool.tile([128, NK], F32, tag="WBS")
        nc.vector.add_range_wrap(WBS[:, 0:256], PB[:, 0:256], 0.0, PI, 2 * PI)
        nc.vector.add_range_wrap(WBS[:, 256:512], PB[:, 256:512], 0.0, PI, 2 * PI)
        WBC = wpool.tile([128, NK], F32, tag="WBC")
        nc.vector.add_range_wrap(WBC[:], PB[:], PI / 2, PI, 2 * PI)
        WA = wpool.tile([128, 2 * NQC], F32, tag="WA")
        nc.vector.add_range_wrap(WA[:, 0:NQC], PA[:], 0.0, PI, 2 * PI)
        nc.vector.add_range_wrap(WA[:, NQC : 2 * NQC], PA[:], PI / 2, PI, 2 * PI)

        # features (bf16); a-side adds omega_m*b1[h] via the ACT bias;
        # c_m*w2[h] folds into the b-side with DVE multiplies that overlap
        # the a-side sin
        BS = fpool.tile([128, NK], BF16, tag="BS")
        nc.scalar.activation(BS[:], WBS[:], AF.Sin)
        BC = fpool.tile([128, NK], BF16, tag="BC")
        nc.scalar.activation(BC[:], WBC[:], AF.Sin)
        A = fpool.tile([128, 2 * NQC], BF16, tag="A")
        nc.scalar.activation(A[:], WA[:], AF.Sin, bias=biasA)
        AS = A[:, 0:NQC]
        AC = A[:, NQC : 2 * NQC]
        BSm = fpool.tile([128, NK], BF16, tag="BSm")
        nc.vector.tensor_scalar_mul(BSm[:], BS[:], cw)
        BCm = fpool.tile([128, NK], BF16, tag="BCm")
        nc.vector.tensor_scalar_mul(BCm[:], BC[:], cw)

        # logits^T[k, q] = sum_f Bfeat[f,k]*Afeat[f,q], 4 k-chunks packed
        # two per PSUM bank
        LA = ps_l.tile([128, 2 * NQC], F32, tag="LA", name="LA")
        LB = ps_l.tile([128, 2 * NQC], F32, tag="LB", name="LB")
        L = [
            LA[:, 0:NQC], LA[:, NQC : 2 * NQC],
            LB[:, 0:NQC], LB[:, NQC : 2 * NQC],
        ]
        for kc in range(4):
            nc.tensor.matmul(
                L[kc], BSm[:, 128 * kc : 128 * (kc + 1)], AC,
                start=True, stop=False,
            )
            nc.tensor.matmul(
                L[kc], BCm[:, 128 * kc : 128 * (kc + 1)], AS,
                start=False, stop=True,
            )

        # exp (no max-subtraction: |logits| <= ~3.2); 2 wide instructions
        E01 = epool.tile([128, 2 * NQC], BF16, tag="E01", name="E01")
        nc.scalar.activation(E01[:], LA[:], AF.Exp)
        E23 = epool.tile([128, 2 * NQC], BF16, tag="E23", name="E23")
        nc.scalar.activation(E23[:], LB[:], AF.Exp)

        def e_chunk(kc, qh_):
            t = E01 if kc < 2 else E23
            c0 = NQC * (kc % 2) + 128 * qh_
            return t[:, c0 : c0 + 128]

        # fused context+rowsum: kctx chunk kc = [keys_chunk | ones], so
        # T[qh][:, 0:128] = context, col 128 = softmax denominator.
        # One PSUM bank per q-half: interleaved accumulation chains must
        # not share a bank.
        T = [
            ps_t.tile([128, 129], F32, tag=f"T{qh_}", name=f"T{qh_}")
            for qh_ in range(2)
        ]
        for kc in range(4):
            for qh_ in range(2):
                nc.tensor.matmul(
                    T[qh_][:],
                    e_chunk(kc, qh_),
                    kctx[:, 129 * kc : 129 * (kc + 1)],
                    start=(kc == 0), stop=(kc == 3),
                )
        for qh_ in range(2):
            rr = epool.tile([128, 1], F32, tag=f"rr{qh_}", name=f"rr{qh_}")
            nc.vector.reciprocal(rr[:], T[qh_][:, 128:129])
            ctxs = epool.tile([128, D], F32, tag=f"ctxs{qh_}", name=f"ctxs{qh_}")
            nc.vector.tensor_scalar_mul(ctxs[:], T[qh_][:, 0:128], rr[:])
            eng = nc.sync if qh_ == 0 else nc.scalar
            eng.dma_start(out[128 * qh_ : 128 * (qh_ + 1), :], ctxs[:])

    nc.compile()
    return nc


def _get_nc():
    global _CACHED_NC
    if _CACHED_NC is None:
        _CACHED_NC = _build_nc()
    return _CACHED_NC


def _in_maps(keys, queries, Wk, Wq, b1, w2):
    keys = np.asarray(keys, np.float32)
    queries = np.asarray(queries, np.float32)
    Wk = np.asarray(Wk, np.float32)
    Wq = np.asarray(Wq, np.float32)
    b1 = np.asarray(b1, np.float32)
    w2 = np.asarray(w2, np.float32)

    om_part = np.repeat(OMEGA, H).astype(np.float32)           # (128,)
    cw_part = np.repeat(COEF, H).astype(np.float32) * np.tile(w2, M)
    bias_part = om_part * np.tile(b1, M)

    # wmat: [WkO | WqO]; WkO[d, 32m+h] = omega_m * Wk[d, h]
    wmat = np.zeros((128, 2 * D), np.float32)
    wmat[:, 0:D] = np.concatenate([o * Wk for o in OMEGA], axis=1)
    wmat[:, D : 2 * D] = np.concatenate([o * Wq for o in OMEGA], axis=1)

    vecs = np.zeros((128, 2), np.float32)
    vecs[:, 0] = cw_part
    vecs[:, 1] = bias_part

    maps = []
    for c in range(8):
        b, half = divmod(c, 2)
        kb = keys[b]  # (512, 128)
        kctx = np.ones((128, 4, 129), np.float32)
        kctx[:, :, :128] = kb.reshape(4, 128, 128).transpose(1, 0, 2)
        maps.append(
            {
                "wmat": wmat.astype(ml_dtypes.bfloat16),
                "kT": kb.T.astype(ml_dtypes.bfloat16),
                "qT": queries[b, NQC * half : NQC * (half + 1)].T.astype(
                    ml_dtypes.bfloat16
                ),
                "kctx": kctx.reshape(128, 4 * 129).astype(ml_dtypes.bfloat16),
                "vecs": vecs,
            }
        )
    return maps


def _run(in_maps, trace=False):
    nc = _get_nc()
    return run_bass_kernel_spmd(nc, in_maps, core_ids=list(range(8)), trace=trace)


def kernel(keys, queries, Wk, Wq, b1, w2, b2):
    res = _run(_in_maps(keys, queries, Wk, Wq, b1, w2))
    outv = np.empty((B, NQ, D), np.float32)
    for c in range(8):
        b, half = divmod(c, 2)
        outv[b, NQC * half : NQC * (half + 1)] = res.results[c]["out"]
    return outv


# revision 14
# speedup vs baseline: 1.0168x; 1.0168x over previous
"""Additive-attention (ContentAttender) Bass kernel for 8 TRN2 NeuronCores.

Problem: B=4, NQ=512, NK=512, D=128, H=32
  logits[b,q,k] = w2 . tanh(qh[b,q] + kh[b,k] + b1) + b2
  out = softmax_k(logits) @ keys

Sharding: data-parallel over (batch x query-half) -> 8 cores, each core
handles one batch's 256 queries vs all 512 keys. No collectives.

Method: the O(NQ*NK*H) tanh tensor is never materialized. Since the score
is a function of a SUM (qh + kh), expand tanh in a separable trig basis:
  tanh(s) ~= sum_m c_m sin(w_m s),  sin(w(a+b)) = sin(wa)cos(wb)+cos(wa)sin(wb)
(M=4 free-fitted frequencies, end-to-end rel err ~2.4e-3, at the bf16
floor). Each core then only evaluates sin/cos features on the small
qh [256,32] / kh [512,32] tensors and contracts the 2M*H=256 feature dim
on the TensorEngine. Phases are built by PE matmuls with omega-prescaled
replicated weights (fp32 PSUM), range-reduced into [-pi,pi] by single
DVE add_range_wrap ops (HW Sin is only accurate to ~|3.3|; cos-tiles get
the +pi/2 as the wrap shift), activated by ACT Sin (b1 folded into the
per-partition bias). Logits come out TRANSPOSED (k on partitions), so
softmax needs no transposes: the rowsum is a ones-column appended to the
keys in the context matmul moving operand. Input DMAs are issued from
five different engine queues so their ~0.6us issue slots overlap; the
Sin->Exp ACT table switch (~1.3us, unavoidable: no table set holds both)
overlaps the logits matmuls.
"""

import contextlib

import numpy as np
import ml_dtypes

import concourse.bass as bass  # noqa: F401
import concourse.mybir as mybir
import concourse.tile as tile
from concourse import bacc
from concourse.bass_utils import run_bass_kernel_spmd

F32 = mybir.dt.float32
BF16 = mybir.dt.bfloat16
AF = mybir.ActivationFunctionType

B, NQ, NK, D, H = 4, 512, 512, 128, 32
NQC = NQ // 2          # queries per core = 256
M = 4                  # trig terms; feature dim = 2*M*H = 256

# fitted tanh(s) ~= sum_m COEF[m] * sin(OMEGA[m] * s)
OMEGA = np.array([0.58658092, 0.58659907, 1.83957819, 3.31890976])
COEF = np.array([1.15549101, -0.0317051, 0.14888519, 0.01860145])

PI = float(np.pi)

_CACHED_NC = None


def _build_nc():
    nc = bacc.Bacc("TRN2", target_bir_lowering=False, debug=False)

    wmat = nc.declare_dram_parameter("wmat", [128, 2 * D], BF16, isOutput=False)
    kTp = nc.declare_dram_parameter("kT", [128, NK], BF16, isOutput=False)
    qTp = nc.declare_dram_parameter("qT", [128, NQC], BF16, isOutput=False)
    kctxp = nc.declare_dram_parameter("kctx", [128, 4 * 129], BF16, isOutput=False)
    vecsp = nc.declare_dram_parameter("vecs", [128, 2], F32, isOutput=False)
    out = nc.declare_dram_parameter("out", [NQC, D], F32, isOutput=True)

    with tile.TileContext(nc) as tc, contextlib.ExitStack() as ctx:
        cpool = ctx.enter_context(tc.tile_pool(name="consts", bufs=1))
        wpool = ctx.enter_context(tc.tile_pool(name="wraps", bufs=1))
        fpool = ctx.enter_context(tc.tile_pool(name="feats", bufs=1))
        epool = ctx.enter_context(tc.tile_pool(name="softmax", bufs=1))
        ps_b = ctx.enter_context(tc.tile_pool(name="ps_b", bufs=1, space="PSUM"))
        ps_a = ctx.enter_context(tc.tile_pool(name="ps_a", bufs=1, space="PSUM"))
        ps_l = ctx.enter_context(tc.tile_pool(name="ps_l", bufs=1, space="PSUM"))
        ps_t = ctx.enter_context(tc.tile_pool(name="ps_t", bufs=1, space="PSUM"))

        # input DMAs issued from three different queues so they overlap
        # (only sync/scalar/gpsimd may issue DMAs)
        kT = cpool.tile([128, NK], BF16, tag="kT")
        nc.sync.dma_start(kT[:], kTp[:])
        wm = cpool.tile([128, 2 * D], BF16, tag="wm")
        nc.scalar.dma_start(wm[:], wmat[:])
        qT = cpool.tile([128, NQC], BF16, tag="qT")
        nc.gpsimd.dma_start(qT[:], qTp[:])
        vecs = cpool.tile([128, 2], F32, tag="vecs")
        nc.gpsimd.dma_start(vecs[:], vecsp[:])
        kctx = cpool.tile([128, 4 * 129], BF16, tag="kctx")
        nc.gpsimd.dma_start(kctx[:], kctxp[:])

        WkO = wm[:, 0:D]
        WqO = wm[:, D : 2 * D]
        cw = vecs[:, 0:1]
        biasA = vecs[:, 1:2]

        # phases: PB[(m,h), k] = omega_m*kh[k,h]; PA[(m,h), q] = omega_m*qh[q,h]
        # a-side first end-to-end: its features gate the logits matmuls
        PA = ps_a.tile([128, NQC], F32, tag="PA", name="PA")
        nc.tensor.matmul(PA[:], WqO, qT, start=True, stop=True)
        PB = ps_b.tile([128, NK], F32, tag="PB", name="PB")
        nc.tensor.matmul(PB[:, 0:256], WkO, kT[:, 0:256], start=True, stop=True)
        nc.tensor.matmul(PB[:, 256:512], WkO, kT[:, 256:512], start=True, stop=True)

        # range-reduce into [-pi, pi]; cos-tiles get +pi/2 shift pre-wrap
        WA = wpool.tile([128, 2 * NQC], F32, tag="WA")
        nc.vector.add_range_wrap(WA[:, 0:NQC], PA[:], 0.0, PI, 2 * PI)
        nc.vector.add_range_wrap(WA[:, NQC : 2 * NQC], PA[:], PI / 2, PI, 2 * PI)
        WBS = wpool.tile([128, NK], F32, tag="WBS")
        nc.vector.add_range_wrap(WBS[:, 0:256], PB[:, 0:256], 0.0, PI, 2 * PI)
        nc.vector.add_range_wrap(WBS[:, 256:512], PB[:, 256:512], 0.0, PI, 2 * PI)
        WBC = wpool.tile([128, NK], F32, tag="WBC")
        nc.vector.add_range_wrap(WBC[:], PB[:], PI / 2, PI, 2 * PI)

        # features (bf16); a-side adds omega_m*b1[h] via the ACT bias;
        # c_m*w2[h] folds into the b-side with DVE multiplies that overlap
        # the remaining sins
        A = fpool.tile([128, 2 * NQC], BF16, tag="A")
        nc.scalar.activation(A[:], WA[:], AF.Sin, bias=biasA)
        AS = A[:, 0:NQC]
        AC = A[:, NQC : 2 * NQC]
        BS = fpool.tile([128, NK], BF16, tag="BS")
        nc.scalar.activation(BS[:], WBS[:], AF.Sin)
        BC = fpool.tile([128, NK], BF16, tag="BC")
        nc.scalar.activation(BC[:], WBC[:], AF.Sin)
        BSm = fpool.tile([128, NK], BF16, tag="BSm")
        nc.vector.tensor_scalar_mul(BSm[:], BS[:], cw)
        BCm = fpool.tile([128, NK], BF16, tag="BCm")
        nc.vector.tensor_scalar_mul(BCm[:], BC[:], cw)

        # logits^T[k, q] = sum_f Bfeat[f,k]*Afeat[f,q], 4 k-chunks packed
        # two per PSUM bank
        LA = ps_l.tile([128, 2 * NQC], F32, tag="LA", name="LA")
        LB = ps_l.tile([128, 2 * NQC], F32, tag="LB", name="LB")
        L = [
            LA[:, 0:NQC], LA[:, NQC : 2 * NQC],
            LB[:, 0:NQC], LB[:, NQC : 2 * NQC],
        ]
        for kc in range(4):
            nc.tensor.matmul(
                L[kc], BSm[:, 128 * kc : 128 * (kc + 1)], AC,
                start=True, stop=False,
            )
            nc.tensor.matmul(
                L[kc], BCm[:, 128 * kc : 128 * (kc + 1)], AS,
                start=False, stop=True,
            )

        # exp (no max-subtraction: |logits| <= ~3.2); 2 wide instructions
        E01 = epool.tile([128, 2 * NQC], BF16, tag="E01", name="E01")
        nc.scalar.activation(E01[:], LA[:], AF.Exp)
        E23 = epool.tile([128, 2 * NQC], BF16, tag="E23", name="E23")
        nc.scalar.activation(E23[:], LB[:], AF.Exp)

        def e_chunk(kc, qh_):
            t = E01 if kc < 2 else E23
            c0 = NQC * (kc % 2) + 128 * qh_
            return t[:, c0 : c0 + 128]

        # fused context+rowsum: kctx chunk kc = [keys_chunk | ones], so
        # T[qh][:, 0:128] = context, col 128 = softmax denominator.
        # One PSUM bank per q-half: interleaved accumulation chains must
        # not share a bank.
        T = [
            ps_t.tile([128, 129], F32, tag=f"T{qh_}", name=f"T{qh_}")
            for qh_ in range(2)
        ]
        for kc in range(4):
            for qh_ in range(2):
                nc.tensor.matmul(
                    T[qh_][:],
                    e_chunk(kc, qh_),
                    kctx[:, 129 * kc : 129 * (kc + 1)],
                    start=(kc == 0), stop=(kc == 3),
                )
        for qh_ in range(2):
            rr = epool.tile([128, 1], F32, tag=f"rr{qh_}", name=f"rr{qh_}")
            nc.vector.reciprocal(rr[:], T[qh_][:, 128:129])
            ctxs = epool.tile([128, D], F32, tag=f"ctxs{qh_}", name=f"ctxs{qh_}")
            nc.vector.tensor_scalar_mul(ctxs[:], T[qh_][:, 0:128], rr[:])
            eng = nc.sync if qh_ == 0 else nc.scalar
            eng.dma_start(out[128 * qh_ : 128 * (qh_ + 1), :], ctxs[:])

    nc.compile()
    return nc


def _get_nc():
    global _CACHED_NC
    if _CACHED_NC is None:
        _CACHED_NC = _build_nc()
    return _CACHED_NC


def _in_maps(keys, queries, Wk, Wq, b1, w2):
    keys = np.asarray(keys, np.float32)
    queries = np.asarray(queries, np.float32)
    Wk = np.asarray(Wk, np.float32)
    Wq = np.asarray(Wq, np.float32)
    b1 = np.asarray(b1, np.float32)
    w2 = np.asarray(w2, np.float32)

    om_part = np.repeat(OMEGA, H).astype(np.float32)           # (128,)
    cw_part = np.repeat(COEF, H).astype(np.float32) * np.tile(w2, M)
    bias_part = om_part * np.tile(b1, M)

    # wmat: [WkO | WqO]; WkO[d, 32m+h] = omega_m * Wk[d, h]
    wmat = np.zeros((128, 2 * D), np.float32)
    wmat[:, 0:D] = np.concatenate([o * Wk for o in OMEGA], axis=1)
    wmat[:, D : 2 * D] = np.concatenate([o * Wq for o in OMEGA], axis=1)

    vecs = np.zeros((128, 2), np.float32)
    vecs[:, 0] = cw_part
    vecs[:, 1] = bias_part

    maps = []
    for c in range(8):
        b, half = divmod(c, 2)
        kb = keys[b]  # (512, 128)
        kctx = np.ones((128, 4, 129), np.float32)
        kctx[:, :, :128] = kb.reshape(4, 128, 128).transpose(1, 0, 2)
        maps.append(
            {
                "wmat": wmat.astype(ml_dtypes.bfloat16),
                "kT": kb.T.astype(ml_dtypes.bfloat16),
                "qT": queries[b, NQC * half : NQC * (half + 1)].T.astype(
                    ml_dtypes.bfloat16
                ),
                "kctx": kctx.reshape(128, 4 * 129).astype(ml_dtypes.bfloat16),
                "vecs": vecs,
            }
        )
    return maps


def _run(in_maps, trace=False):
    nc = _get_nc()
    return run_bass_kernel_spmd(nc, in_maps, core_ids=list(range(8)), trace=trace)


def kernel(keys, queries, Wk, Wq, b1, w2, b2):
    res = _run(_in_maps(keys, queries, Wk, Wq, b1, w2))
    outv = np.empty((B, NQ, D), np.float32)
    for c in range(8):
        b, half = divmod(c, 2)
        outv[b, NQC * half : NQC * (half + 1)] = res.results[c]["out"]
    return outv
